# revision 1
# baseline (speedup 1.0000x reference)
"""Trainium2 Bass kernel for the 5-layer LSTM + FC head (nn_LstmMemoryPredict).

Data-parallel over 8 NeuronCores: batch 256 -> 32 per core; LSTM layers run as
a 5-deep wavefront per core; weights replicated. Self-contained.
"""
import sys
sys.path.insert(0, "/opt/trn_rl_repo")

import numpy as np
import concourse.bass as bass
import concourse.bacc as bacc
import concourse.mybir as mybir
from concourse.tile import TileContext
from concourse.mybir import AluOpType, ActivationFunctionType

F32 = mybir.dt.float32
BF16 = mybir.dt.bfloat16

H, L, D, BC = 32, 5, 64, 32     # hidden, layers, input dim, batch/core
F = L * BC                      # 160: free size of one wavefront step
PRO = L - 1                     # 4 prologue steps
GATE_PERM = np.r_[0:64, 96:128, 64:96]  # i,f,g,o -> i,f,o,g


def build(T: int, CHUNK: int, mm_dt=BF16):
    assert T % CHUNK == 0
    NB = T // CHUNK
    nc = bacc.Bacc(None, target_bir_lowering=False, debug=False)

    xt_d = nc.dram_tensor("xt", [D, (T + PRO) * BC], mm_dt, kind="ExternalInput")
    w0_d = nc.dram_tensor("w0t", [D, 128], mm_dt, kind="ExternalInput")
    wcat_d = nc.dram_tensor("wcat", [H, 9 * 128], mm_dt, kind="ExternalInput")
    b5_d = nc.dram_tensor("b5", [L, 128], F32, kind="ExternalInput")
    oneh_d = nc.dram_tensor("oneh", [L, 2 * F], F32, kind="ExternalInput")
    fcw_d = nc.dram_tensor("fcw", [H + 1, 1], F32, kind="ExternalInput")
    out_d = nc.dram_tensor("out", [1, T * BC], F32, kind="ExternalOutput")

    with TileContext(nc) as tc:
        with (
            tc.tile_pool(name="wpool", bufs=1) as wpool,
            tc.tile_pool(name="state", bufs=1) as state,
            tc.tile_pool(name="xpool", bufs=2) as xpool,
            tc.tile_pool(name="rpool", bufs=2) as rpool,
            tc.tile_pool(name="opool", bufs=2) as opool,
            tc.tile_pool(name="spool", bufs=3) as spool,
            tc.tile_pool(name="zpool", bufs=5, space="PSUM") as zpool,
            tc.tile_pool(name="fcpool", bufs=2, space="PSUM") as fcpool,
        ):
            # ---- constants / weights ----
            w0 = wpool.tile([D, 128], mm_dt)
            wcat = wpool.tile([H, 9 * 128], mm_dt)
            b5 = wpool.tile([L, 128], F32)
            oneh = wpool.tile([L, 2 * F], F32)
            fcw = wpool.tile([H + 1, 1], F32)
            xpro = wpool.tile([D, PRO * BC], mm_dt)
            nc.sync.dma_start(w0[:], w0_d[:, :])
            nc.sync.dma_start(wcat[:], wcat_d[:, :])
            nc.sync.dma_start(b5[:], b5_d[:, :])
            nc.sync.dma_start(oneh[:], oneh_d[:, :])
            nc.sync.dma_start(fcw[:], fcw_d[:, :])
            nc.sync.dma_start(xpro[:], xt_d[:, 0:PRO * BC])

            # ---- persistent state ----
            hst = state.tile([H, F], mm_dt)        # h for 5 layers
            u = state.tile([H, 2 * F], F32)        # cols 0:F g, F:2F c
            nc.gpsimd.memset(hst[:], 0.0)
            nc.gpsimd.memset(u[:], 0.0)

            # wcat block index: wh_l at 2l, wx_l at 2l-1 (l>=1)
            def wh(l):
                return wcat[:, (2 * l) * 128:(2 * l + 1) * 128]

            def wx(l):
                return wcat[:, (2 * l - 1) * 128:(2 * l) * 128]

            def emit_step(zb, k2, x_mv, ring=None, ring_col=0, mask_from=None):
                z = zb[:, k2 * F:(k2 + 1) * F]
                if k2 == 0:  # bias for both steps of this bank
                    nc.tensor.matmul(zb[:, :], b5[:], oneh[:], start=True,
                                     stop=False, skip_group_check=True)
                # layer-0 input projection from x
                nc.tensor.matmul(z[:, 0:BC], w0[:], x_mv, start=False,
                                 stop=False, skip_group_check=True)
                # recurrent + inter-layer projections (all consume hst)
                for l in range(L):
                    nc.tensor.matmul(z[:, l * BC:(l + 1) * BC], wh(l),
                                     hst[:, l * BC:(l + 1) * BC], start=False,
                                     stop=False, skip_group_check=True)
                for l in range(1, L):
                    last = l == L - 1
                    nc.tensor.matmul(z[:, l * BC:(l + 1) * BC], wx(l),
                                     hst[:, (l - 1) * BC:l * BC], start=False,
                                     stop=last, skip_group_check=True)
                # activations: everything lands on partitions 0-31,
                # gates packed along the free dim (ACT may remap bases)
                p = spool.tile([H, 2 * F], F32, tag="p")      # i | f
                o = spool.tile([H, F], F32, tag="o")
                nc.scalar.activation(p[:, 0:F], z[0:H, :],
                                     ActivationFunctionType.Sigmoid)
                nc.scalar.activation(p[:, F:2 * F], z[H:2 * H, :],
                                     ActivationFunctionType.Sigmoid)
                nc.scalar.activation(o[:, :], z[2 * H:3 * H, :],
                                     ActivationFunctionType.Sigmoid)
                nc.scalar.activation(u[:, 0:F], z[3 * H:4 * H, :],
                                     ActivationFunctionType.Tanh)
                # cell update: u cols 0:F = g, F:2F = c (all base partition 0)
                a = spool.tile([H, 2 * F], F32, tag="a")
                nc.vector.tensor_tensor(a[:, :], p[:, :], u[:, :],
                                        AluOpType.mult)
                nc.vector.tensor_tensor(u[:, F:2 * F], a[:, 0:F], a[:, F:2 * F],
                                        AluOpType.add)
                tcn = spool.tile([H, F], F32, tag="tc")
                nc.scalar.activation(tcn[:, :], u[:, F:2 * F],
                                     ActivationFunctionType.Tanh)
                nc.vector.tensor_tensor(hst[:, :], o[:, :], tcn[:, :],
                                        AluOpType.mult)
                if ring is not None:
                    nc.vector.tensor_tensor(
                        ring[0:H, ring_col:ring_col + BC],
                        o[:, (L - 1) * BC:F],
                        tcn[:, (L - 1) * BC:F], AluOpType.mult)
                if mask_from is not None:
                    nc.gpsimd.memset(u[:, F + mask_from * BC:2 * F], 0.0)
                    nc.gpsimd.memset(hst[:, mask_from * BC:F], 0.0)

            # ---- prologue: wavefront warm-up, steps s=0..3 ----
            zb = None
            for s in range(PRO):
                if s % 2 == 0:
                    zb = zpool.tile([128, 2 * F], F32, tag="zb")
                emit_step(zb, s % 2, xpro[:, s * BC:(s + 1) * BC],
                          mask_from=s + 1)

            # ---- main loop over chunks ----
            FCN = CHUNK * BC // 512  # FC matmuls per chunk
            with tc.For_i(0, NB) as i:
                xb = xpool.tile([D, CHUNK * BC], mm_dt)
                nc.sync.dma_start(
                    xb[:], xt_d[:, bass.ds(i * (CHUNK * BC) + PRO * BC,
                                           CHUNK * BC)])
                ring = rpool.tile([H + 1, CHUNK * BC], F32)
                nc.gpsimd.memset(ring[H:H + 1, :], 1.0)
                ost = opool.tile([1, CHUNK * BC], F32)

                def fc_block(q):
                    fps = fcpool.tile([1, 512], F32, tag="fps")
                    nc.tensor.matmul(fps[:, :], fcw[:],
                                     ring[:, q * 512:(q + 1) * 512],
                                     start=True, stop=True,
                                     skip_group_check=True)
                    nc.vector.tensor_copy(ost[:, q * 512:(q + 1) * 512],
                                          fps[:, :])

                for sl in range(CHUNK):
                    if sl % 2 == 0:
                        zb = zpool.tile([128, 2 * F], F32, tag="zb")
                    emit_step(zb, sl % 2, xb[:, sl * BC:(sl + 1) * BC],
                              ring=ring, ring_col=sl * BC)
                    if sl % 16 == 15 and sl >= 31:
                        fc_block(sl // 16 - 1)
                fc_block(FCN - 1)
                nc.sync.dma_start(out_d[:, bass.ds(i * (CHUNK * BC),
                                                   CHUNK * BC)], ost[:, :])

    nc.compile()
    return nc


# ---------------- host-side packing ----------------

def prep_weights(W_ih0, W_ih_rest, W_hh, b_ih, b_hh, W_fc, b_fc, mm_np):
    p = GATE_PERM
    w0t = np.ascontiguousarray(W_ih0[p].T).astype(mm_np)           # [64,128]
    blocks = [W_hh[0][p].T]
    for l in range(1, L):
        blocks.append(W_ih_rest[l - 1][p].T)
        blocks.append(W_hh[l][p].T)
    wcat = np.concatenate(blocks, axis=1).astype(mm_np)            # [32,1152]
    b5 = (b_ih + b_hh)[:, p].astype(np.float32)                    # [5,128]
    oneh = np.zeros((L, 2 * F), np.float32)
    for k in range(L):
        for k2 in range(2):
            oneh[k, k2 * F + k * BC:k2 * F + (k + 1) * BC] = 1.0
    fcw = np.concatenate([W_fc.reshape(H, 1), b_fc.reshape(1, 1)],
                         axis=0).astype(np.float32)                # [33,1]
    return {"w0t": w0t, "wcat": wcat, "b5": b5, "oneh": oneh, "fcw": fcw}


def prep_x_core(x_core, T, mm_np):
    # x_core [BC, T, D] fp32 -> xt [64, (T+PRO)*BC], j = t*BC + b, zero tail
    xt = np.zeros((D, (T + PRO) * BC), np.float32)
    xt[:, :T * BC] = x_core.transpose(2, 1, 0).reshape(D, T * BC)
    return xt.astype(mm_np)


# ---------------- public entry point ----------------
T_FULL, CHUNK_FULL, N_CORES = 2048, 32, 8
_NC_CACHE = {}


def _get_nc():
    if "nc" not in _NC_CACHE:
        _NC_CACHE["nc"] = build(T_FULL, CHUNK_FULL)
    return _NC_CACHE["nc"]


def kernel(x, W_ih0, W_ih_rest, W_hh, b_ih, b_hh, W_fc, b_fc):
    import ml_dtypes
    from concourse.bass_utils import run_bass_kernel_spmd
    mm_np = ml_dtypes.bfloat16
    nc = _get_nc()
    w = prep_weights(np.asarray(W_ih0), np.asarray(W_ih_rest), np.asarray(W_hh),
                     np.asarray(b_ih), np.asarray(b_hh), np.asarray(W_fc),
                     np.asarray(b_fc), mm_np)
    x = np.asarray(x)
    B = x.shape[0]
    in_maps = []
    for c in range(N_CORES):
        xs = x[c * BC:(c + 1) * BC]
        in_maps.append(dict(w, xt=prep_x_core(xs, T_FULL, mm_np)))
    res = run_bass_kernel_spmd(nc, in_maps, core_ids=list(range(N_CORES)))
    outs = []
    for c in range(N_CORES):
        o = res.results[c]["out"].reshape(T_FULL, BC).T[:, :, None]
        outs.append(o)
    return np.concatenate(outs, axis=0).astype(np.float32)

